# revision 8
# baseline (speedup 1.0000x reference)
"""ContextQueryAttention (BiDAF-style) Trainium2 kernel, 8-core data-parallel.

Math (per batch):
  s[i,j]  = wq.q_j + wc.c_i + sum_d c_id * wcq_d * q_jd          (L1 x L2)
  s1      = softmax_i(s * mq_j + (1-mq_j)*NEG)                   (softmax over i)
  s2      = softmax_i(s * mp_i + (1-mp_i)*NEG)
  a       = s1 @ Q                 (L1 x D)
  b       = (s1 @ s2^T) @ C  ==  s1 @ (s2^T @ C)   <- reassociated, no L1xL1
  out     = [C, a, C*a, C*b]                                      (L1 x 4D)

Kernel structure (v3 — engine-balanced):
 - qwq_j is constant along the softmax axis (i) in both softmaxes -> cancels.
 - wc folded into the shared query-side operand qtw'[d,j] = wcq_d*q[j,d] + wc_d
   so BOTH score matmuls produce  psum = dot + cwc_i  directly:
     E1 layout [j part, i free]: stationary qtw1 = qtw' * mq_j, moving ct
     E2 layout [i part, j free]: stationary ct tile,  moving qtw'
 - masks folded into matmul operands -> both exp passes are maskless 1024-wide
   ACTs straight from 2-bank PSUM strips (scalar cost (N+352)/1.2ns):
     e1 = exp(mq_j*(dot+cwc))     (masked col -> exp(0)=1 -> uniform 1/L1)
     e2 = exp(dot+cwc) unmasked; mp folded into the t matmul's C operand
 - t matmul moving operand c1m[i,:] = [mp_i*c_i | mp_i | 1] gives t, z2 AND
   z1 raw column sums in one accumulation; z1 = mq_j*(colsum-L1)+L1 fixes
   masked columns. No reduce ops, no ACT accumulators.
 - output assembled in SBUF as 4-tile groups [c|a|c*a|c*b] (strided vector
   ops over [128,4,*] views, c copied by gpsimd), one contiguous-row DMA per
   512 rows. No DRAM->DRAM context copy.
 - input DMA issue spread across sync/scalar/gpsimd queues; output DMAs on
   sync (descriptor gen costs ~0.8us per issue on the issuing queue).
 - PE pre-warm dummy transposes keep the HAM clock gate at 2.4GHz during the
   input-DMA window (cold default is 1.2GHz, ~3.4us ramp).
 - matmul operands bf16, accumulation f32 in PSUM.
"""

import numpy as np

import concourse.bass as bass
import concourse.mybir as mybir
import concourse.tile as tile
from concourse import bacc
from concourse import bass_utils
from concourse.masks import make_identity

F32 = mybir.dt.float32
BF16 = mybir.dt.bfloat16
EXP = mybir.ActivationFunctionType.Exp
ADD = mybir.AluOpType.add
MULT = mybir.AluOpType.mult

B, L1, L2, D = 16, 2048, 512, 128
NCORES = 8
BPC = B // NCORES          # batches per core
NT1 = L1 // 128            # 16 i-tiles
NT2 = L2 // 128            # 4  j-tiles
NWARM = 40                 # PE pre-warm dummy matmuls


def _build_program(dbg=False):
    nc = bacc.Bacc("TRN2", target_bir_lowering=False, debug=False)

    ctx_d = nc.dram_tensor("context", [BPC, L1, D], F32, kind="ExternalInput").ap()
    qry_d = nc.dram_tensor("query", [BPC, L2, D], F32, kind="ExternalInput").ap()
    w_d = nc.dram_tensor("w", [3, D], F32, kind="ExternalInput").ap()
    mp_d = nc.dram_tensor("mask_p", [BPC, L1], F32, kind="ExternalInput").ap()
    mq_d = nc.dram_tensor("mask_q", [BPC, L2], F32, kind="ExternalInput").ap()
    out_d = nc.dram_tensor("out", [BPC, L1, 4 * D], F32, kind="ExternalOutput").ap()

    with tile.TileContext(nc) as tc:
        with (
            tc.tile_pool(name="const", bufs=1) as const,
            tc.tile_pool(name="big", bufs=2) as big,
            tc.tile_pool(name="work", bufs=2) as work,
            tc.tile_pool(name="outp", bufs=3) as outp,
            tc.tile_pool(name="strips", bufs=2, space="PSUM") as strips,
            tc.tile_pool(name="small", bufs=2, space="PSUM") as small,
        ):
            ident_b = const.tile([128, 128], BF16)
            make_identity(nc, ident_b)
            w_sb = const.tile([128, 3], F32)  # cols: wq, wc, wcq
            nc.sync.dma_start(out=w_sb, in_=w_d.rearrange("k d -> d k"))

            # PE pre-warm: keep the HAM clock gate open during input DMA
            warm_ps = small.tile([128, 128], BF16, tag="acc", name="warm")
            for _ in range(NWARM):
                nc.tensor.transpose(warm_ps, ident_b, ident_b)

            S = [dict() for _ in range(BPC)]  # per-batch tile state

            def ph_dma(b):
                s = S[b]
                engs = ([nc.sync, nc.scalar, nc.gpsimd, nc.sync]
                        if b == 0 else
                        [nc.gpsimd, nc.sync, nc.scalar, nc.gpsimd])
                s["qn"] = work.tile([128, NT2, 128], F32, tag="qn", name=f"qn{b}")
                engs[0].dma_start(
                    out=s["qn"], in_=qry_d[b].rearrange("(t p) d -> p t d", p=128)
                )
                s["mp"] = work.tile([128, NT1], F32, tag="mp", name=f"mp{b}")
                engs[1].dma_start(
                    out=s["mp"], in_=mp_d[b].rearrange("(t p) -> p t", p=128)
                )
                s["mq"] = work.tile([128, NT2], F32, tag="mq", name=f"mq{b}")
                engs[2].dma_start(
                    out=s["mq"], in_=mq_d[b].rearrange("(t p) -> p t", p=128)
                )
                s["mqr"] = work.tile([1, L2], F32, tag="mqr", name=f"mqr{b}")
                engs[3].dma_start(out=s["mqr"], in_=mq_d[b : b + 1, :])
                s["c1"] = big.tile([128, NT1, 128], F32, tag="c1", name=f"c1_{b}")
                ctx_r = ctx_d[b].rearrange("(t p) d -> p t d", p=128)
                for n in range(4):
                    engs[n].dma_start(
                        out=s["c1"][:, 4 * n : 4 * (n + 1), :],
                        in_=ctx_r[:, 4 * n : 4 * (n + 1), :],
                    )

            def ph_qprep(b):
                s = S[b]
                qnb = work.tile([128, NT2, 128], BF16, tag="qnb")
                nc.vector.tensor_copy(qnb, s["qn"])
                s["qnb"] = qnb
                ps = small.tile([128, 4, 128], BF16, tag="acc")
                for jt in range(NT2):
                    nc.tensor.transpose(ps[:, jt, :], qnb[:, jt, :], ident_b)
                qt = work.tile([128, NT2, 128], BF16, tag="qt")
                nc.vector.tensor_copy(qt, ps)
                # qtw' = qt*wcq + wc  (shared moving operand / e2)
                qtw = work.tile([128, NT2, 128], BF16, tag="qtw")
                nc.vector.tensor_scalar(
                    out=qtw, in0=qt, scalar1=w_sb[:, 2:3], scalar2=w_sb[:, 1:2],
                    op0=MULT, op1=ADD,
                )
                s["qtw"] = qtw
                # qtw1 = qtw' * mq_j  (e1 stationary; mq along free axis)
                mqb = work.tile([128, L2], F32, tag="mqb", name=f"mqb{b}")
                nc.gpsimd.partition_broadcast(mqb, s["mqr"])
                qtw1 = work.tile([128, NT2, 128], BF16, tag="qtw1")
                nc.vector.tensor_tensor(
                    qtw1.rearrange("p t d -> p (t d)"),
                    qtw.rearrange("p t d -> p (t d)"), mqb, MULT,
                )
                s["qtw1"] = qtw1

            def ph_cprep(b):
                s = S[b]
                cb = big.tile([128, NT1, 128], BF16, tag="cb")
                nc.vector.tensor_copy(cb, s["c1"])
                ct = big.tile([128, NT1, 128], BF16, tag="ct")
                for n in range(4):
                    ps = small.tile([128, 4, 128], BF16, tag="acc")
                    for k in range(4):
                        nc.tensor.transpose(ps[:, k, :], cb[:, 4 * n + k, :], ident_b)
                    nc.vector.tensor_copy(ct[:, 4 * n : 4 * (n + 1), :], ps)
                s["ct"] = ct
                # c1m = [mp_i * c | mp_i | 1]  (mask_p + z2 + z1-colsum operand)
                c1m = big.tile([128, NT1, 130], BF16, tag="c1m")
                for it in range(NT1):
                    nc.vector.tensor_scalar_mul(
                        c1m[:, it, 0:128], cb[:, it, :], s["mp"][:, it : it + 1]
                    )
                nc.vector.tensor_copy(
                    c1m[:, :, 128:129].rearrange("p a b -> p (a b)"), s["mp"]
                )
                nc.vector.memset(c1m[:, :, 129:130], 1.0)
                s["c1m"] = c1m

            def ph_e2(b):
                # e2[i,j] = exp(dot + cwc_i), unmasked (mp applied via c1m)
                s = S[b]
                e2n = big.tile([128, NT1, L2], BF16, tag="e2n")
                for g in range(NT1 // 2):
                    st = strips.tile([128, 2, 512], F32, tag="strip")
                    for k in range(2):
                        nc.tensor.matmul(
                            st[:, k, :], s["ct"][:, 2 * g + k, :],
                            s["qtw"].rearrange("p t d -> p (t d)"),
                            start=True, stop=True,
                        )
                    nc.scalar.activation(e2n[:, 2 * g : 2 * g + 2, :], st, EXP)
                s["e2n"] = e2n

            def ph_e1(b):
                # e1[j,i] = exp(mq_j * (dot + cwc_i)); masked col -> 1 (uniform)
                s = S[b]
                e1 = big.tile([128, NT2, L1], BF16, tag="e1")
                for jt in range(NT2):
                    for h in range(2):
                        st = strips.tile([128, 2, 512], F32, tag="strip")
                        for k in range(2):
                            m = 2 * h + k
                            nc.tensor.matmul(
                                st[:, k, :], s["qtw1"][:, jt, :],
                                s["ct"][:, 4 * m : 4 * (m + 1), :],
                                start=True, stop=True,
                            )
                        nc.scalar.activation(
                            e1[:, jt, 1024 * h : 1024 * (h + 1)],
                            st.rearrange("p a b -> p (a b)"), EXP,
                        )
                s["e1"] = e1

            def ph_t(b):
                s = S[b]
                rhs_ab = work.tile([128, NT2, 256], BF16, tag="rhs_ab")
                for jt in range(NT2):
                    pst = small.tile([128, 130], F32, tag="acc")
                    for it in range(NT1):
                        nc.tensor.matmul(
                            pst, s["e2n"][:, it, jt * 128 : (jt + 1) * 128],
                            s["c1m"][:, it, :],
                            start=(it == 0), stop=(it == NT1 - 1),
                        )
                    # z1_j = mq_j*(colsum_j - L1) + L1 ; colsum in pst[:,129]
                    z1 = work.tile([128, 1], F32, tag="z1")
                    nc.vector.scalar_tensor_tensor(
                        out=z1, in0=pst[:, 129:130], scalar=-float(L1),
                        in1=s["mq"][:, jt : jt + 1], op0=ADD, op1=MULT,
                    )
                    nc.vector.tensor_scalar_add(z1, z1, float(L1))
                    rz1 = work.tile([128, 1], F32, tag="rz1")
                    nc.vector.reciprocal(rz1, z1)
                    rz2 = work.tile([128, 1], F32, tag="rz2")
                    nc.vector.reciprocal(rz2, pst[:, 128:129])
                    rz12 = work.tile([128, 1], F32, tag="rz12")
                    nc.vector.tensor_mul(rz12, rz2, rz1)
                    nc.vector.tensor_scalar_mul(
                        rhs_ab[:, jt, 128:256], pst[:, 0:128], rz12
                    )
                    nc.vector.tensor_scalar_mul(
                        rhs_ab[:, jt, 0:128], s["qnb"][:, jt, :], rz1
                    )
                s["rhs_ab"] = rhs_ab

            def ph_ab(b):
                s = S[b]
                for g in range(NT1 // 4):
                    psab = small.tile([128, 4, 256], F32, tag="acc")
                    # groups must accumulate consecutively: a start=True into a
                    # bank clears has_written for the whole bank, so interleaving
                    # two accumulating groups in one bank drops contributions
                    for gi in range(4):
                        it = 4 * g + gi
                        for jt in range(NT2):
                            nc.tensor.matmul(
                                psab[:, gi, :],
                                s["e1"][:, jt, it * 128 : (it + 1) * 128],
                                s["rhs_ab"][:, jt, :],
                                start=(jt == 0), stop=(jt == NT2 - 1),
                            )
                    o_sb = outp.tile([128, 4, 512], F32, tag="o_sb")
                    c_sl = s["c1"][:, 4 * g : 4 * (g + 1), :]
                    nc.gpsimd.tensor_copy(o_sb[:, :, 0:128], c_sl)
                    nc.vector.tensor_copy(o_sb[:, :, 128:256], psab[:, :, 0:128])
                    nc.vector.tensor_tensor(
                        o_sb[:, :, 256:384], c_sl, psab[:, :, 0:128], MULT
                    )
                    nc.vector.tensor_tensor(
                        o_sb[:, :, 384:512], c_sl, psab[:, :, 128:256], MULT
                    )
                    nc.sync.dma_start(
                        out=out_d[b, 512 * g : 512 * (g + 1), :].rearrange(
                            "(t p) m -> p t m", p=128
                        ),
                        in_=o_sb,
                    )

            def ph_dbg(b):
                if not (dbg and b == 0):
                    return
                s = S[b]
                for name, key in [
                    ("dbg_e1", "e1"), ("dbg_e2n", "e2n"),
                    ("dbg_rhs_ab", "rhs_ab"), ("dbg_ct", "ct"), ("dbg_qtw", "qtw"),
                    ("dbg_qtw1", "qtw1"), ("dbg_c1m", "c1m"),
                ]:
                    src = s[key]
                    dd = nc.dram_tensor(
                        name, list(src.shape), src.dtype, kind="ExternalOutput"
                    ).ap()
                    nc.sync.dma_start(out=dd, in_=src)

            # interleaved emission: scheduler always has cross-batch slack
            ph_dma(0); ph_dma(1)
            ph_qprep(0); ph_cprep(0)
            ph_e2(0); ph_qprep(1)
            ph_e1(0); ph_cprep(1)
            ph_t(0); ph_e2(1)
            ph_ab(0); ph_e1(1)
            ph_t(1); ph_ab(1)
            ph_dbg(0)

    nc.compile()
    return nc


_NC = None


def _get_nc():
    global _NC
    if _NC is None:
        _NC = _build_program()
    return _NC


def _make_in_maps(inputs):
    context, query, w = inputs["context"], inputs["query"], inputs["w"]
    w2 = np.ascontiguousarray(np.asarray(w).reshape(3, D).astype(np.float32))
    mp = np.asarray(inputs["mask_p"]).astype(np.float32)
    mq = np.asarray(inputs["mask_q"]).astype(np.float32)
    in_maps = []
    for c in range(NCORES):
        sl = slice(c * BPC, (c + 1) * BPC)
        in_maps.append(
            {
                "context": np.ascontiguousarray(context[sl]),
                "query": np.ascontiguousarray(query[sl]),
                "w": w2,
                "mask_p": np.ascontiguousarray(mp[sl]),
                "mask_q": np.ascontiguousarray(mq[sl]),
            }
        )
    return in_maps


def kernel(context, query, w, mask_p, mask_q):
    nc = _get_nc()
    in_maps = _make_in_maps(
        {"context": context, "query": query, "w": w, "mask_p": mask_p, "mask_q": mask_q}
    )
    res = bass_utils.run_bass_kernel_spmd(nc, in_maps, core_ids=list(range(NCORES)))
    return np.concatenate([res.results[c]["out"] for c in range(NCORES)], axis=0)


# revision 9
# speedup vs baseline: 1.1401x; 1.1401x over previous
"""ContextQueryAttention (BiDAF-style) Trainium2 kernel, 8-core data-parallel.

Math (per batch):
  s[i,j]  = wq.q_j + wc.c_i + sum_d c_id * wcq_d * q_jd          (L1 x L2)
  s1      = softmax_i(s * mq_j + (1-mq_j)*NEG)                   (softmax over i)
  s2      = softmax_i(s * mp_i + (1-mp_i)*NEG)
  a       = s1 @ Q                 (L1 x D)
  b       = (s1 @ s2^T) @ C  ==  s1 @ (s2^T @ C)   <- reassociated, no L1xL1
  out     = [C, a, C*a, C*b]                                      (L1 x 4D)

Kernel structure (v4 — engine-balanced):
 - qwq_j is constant along the softmax axis (i) in both softmaxes -> cancels.
 - wc folded into the shared query-side operand qtw'[d,j] = wcq_d*q[j,d] + wc_d
   so BOTH score matmuls produce  psum = dot + cwc_i  directly:
     E1 layout [j part, i free]: stationary qtw1 = qtw' * mq_j, moving ct
     E2 layout [i part, j free]: stationary ct tile,  moving qtw'
 - masks folded into matmul operands -> both exp passes are maskless 1024-wide
   ACTs straight from 2-bank PSUM strips (scalar cost (N+352)/1.2ns):
     e1 = exp(mq_j*(dot+cwc))     (masked col -> exp(0)=1 -> uniform 1/L1)
     e2 = exp(dot+cwc) unmasked; mp folded into the t matmul's C operand
 - t matmul moving operand c1m[i,:] = [mp_i*c_i | mp_i | 1] gives t, z2 AND
   z1 raw column sums in one accumulation; z1 = mq_j*(colsum-L1)+L1 fixes
   masked columns. No reduce ops, no ACT accumulators.
 - PSUM accumulation groups never interleave within a bank (start=True clears
   has_written for the whole bank).
 - out c-section DMAed straight from the c1 SBUF tile early; [a|c*a|c*b]
   assembled in SBUF as 4-tile groups via strided vector ops, one
   contiguous-row DMA per 512 rows.
 - masks arrive host-pretransposed in one tiny contiguous aux tensor (a
   strided (t p)->p t DMA would generate 4-byte packets and stall the queue).
 - input DMA issue spread across sync/scalar/gpsimd queues (descriptor gen
   costs ~0.7us per issue on the issuing queue); output DMAs on sync.
 - PE pre-warm dummy transposes keep the HAM clock gate at 2.4GHz during the
   input-DMA window (cold default is 1.2GHz, ~3.4us ramp).
 - matmul operands bf16, accumulation f32 in PSUM.
"""

import numpy as np

import concourse.bass as bass
import concourse.mybir as mybir
import concourse.tile as tile
from concourse import bacc
from concourse import bass_utils
from concourse.masks import make_identity

F32 = mybir.dt.float32
BF16 = mybir.dt.bfloat16
EXP = mybir.ActivationFunctionType.Exp
ADD = mybir.AluOpType.add
MULT = mybir.AluOpType.mult

B, L1, L2, D = 16, 2048, 512, 128
NCORES = 8
BPC = B // NCORES          # batches per core
NT1 = L1 // 128            # 16 i-tiles
NT2 = L2 // 128            # 4  j-tiles
NWARM = 48                 # PE pre-warm dummy matmuls


def _build_program(dbg=False):
    nc = bacc.Bacc("TRN2", target_bir_lowering=False, debug=False)

    ctx_d = nc.dram_tensor("context", [BPC, L1, D], F32, kind="ExternalInput").ap()
    qry_d = nc.dram_tensor("query", [BPC, L2, D], F32, kind="ExternalInput").ap()
    # aux[b] = [mp_t (16) | mq_t (4) | w^T (3)] as [128, 23] f32, host-packed
    aux_d = nc.dram_tensor("aux", [BPC, 128, NT1 + NT2 + 3], F32,
                           kind="ExternalInput").ap()
    mqr_d = nc.dram_tensor("mq_row", [BPC, L2], F32, kind="ExternalInput").ap()
    out_d = nc.dram_tensor("out", [BPC, L1, 4 * D], F32, kind="ExternalOutput").ap()

    with tile.TileContext(nc) as tc:
        with (
            tc.tile_pool(name="const", bufs=1) as const,
            tc.tile_pool(name="big", bufs=2) as big,
            tc.tile_pool(name="work", bufs=2) as work,
            tc.tile_pool(name="outp", bufs=3) as outp,
            tc.tile_pool(name="strips", bufs=2, space="PSUM") as strips,
            tc.tile_pool(name="small", bufs=2, space="PSUM") as small,
        ):
            ident_b = const.tile([128, 128], BF16)
            make_identity(nc, ident_b)

            # PE pre-warm: keep the HAM clock gate open during input DMA
            warm_ps = small.tile([128, 128], BF16, tag="acc", name="warm")
            for _ in range(NWARM):
                nc.tensor.transpose(warm_ps, ident_b, ident_b)

            S = [dict() for _ in range(BPC)]  # per-batch tile state

            def ph_dma(b):
                s = S[b]
                engs = ([nc.sync, nc.scalar, nc.gpsimd]
                        if b == 0 else
                        [nc.gpsimd, nc.sync, nc.scalar])
                s["qn"] = work.tile([128, NT2, 128], F32, tag="qn", name=f"qn{b}")
                engs[0].dma_start(
                    out=s["qn"], in_=qry_d[b].rearrange("(t p) d -> p t d", p=128)
                )
                s["aux"] = work.tile([128, NT1 + NT2 + 3], F32, tag="aux",
                                     name=f"aux{b}")
                engs[1].dma_start(out=s["aux"], in_=aux_d[b])
                s["mqr"] = work.tile([1, L2], F32, tag="mqr", name=f"mqr{b}")
                engs[2].dma_start(out=s["mqr"], in_=mqr_d[b : b + 1, :])
                s["mp"] = s["aux"][:, 0:NT1]
                s["mq"] = s["aux"][:, NT1 : NT1 + NT2]
                s["wc"] = s["aux"][:, NT1 + NT2 + 1 : NT1 + NT2 + 2]
                s["wcq"] = s["aux"][:, NT1 + NT2 + 2 : NT1 + NT2 + 3]
                s["c1"] = big.tile([128, NT1, 128], F32, tag="c1", name=f"c1_{b}")
                ctx_r = ctx_d[b].rearrange("(t p) d -> p t d", p=128)
                for n in range(4):
                    engs[n % 3].dma_start(
                        out=s["c1"][:, 4 * n : 4 * (n + 1), :],
                        in_=ctx_r[:, 4 * n : 4 * (n + 1), :],
                    )
                # context section of out streams straight from SBUF
                for half in range(2):
                    engs[half].dma_start(
                        out=out_d[b, half * 1024 : (half + 1) * 1024, 0:128]
                        .rearrange("(t p) m -> p t m", p=128),
                        in_=s["c1"][:, 8 * half : 8 * (half + 1), :],
                    )

            def ph_qprep(b):
                s = S[b]
                qnb = work.tile([128, NT2, 128], BF16, tag="qnb")
                nc.vector.tensor_copy(qnb, s["qn"])
                s["qnb"] = qnb
                ps = small.tile([128, 4, 128], BF16, tag="acc")
                for jt in range(NT2):
                    nc.tensor.transpose(ps[:, jt, :], qnb[:, jt, :], ident_b)
                qt = work.tile([128, NT2, 128], BF16, tag="qt")
                nc.vector.tensor_copy(qt, ps)
                # qtw' = qt*wcq + wc  (shared moving operand / e2)
                qtw = work.tile([128, NT2, 128], BF16, tag="qtw")
                nc.vector.tensor_scalar(
                    out=qtw, in0=qt, scalar1=s["wcq"], scalar2=s["wc"],
                    op0=MULT, op1=ADD,
                )
                s["qtw"] = qtw
                # qtw1 = qtw' * mq_j  (e1 stationary; mq along free axis)
                mqb = work.tile([128, L2], F32, tag="mqb", name=f"mqb{b}")
                nc.gpsimd.partition_broadcast(mqb, s["mqr"])
                qtw1 = work.tile([128, NT2, 128], BF16, tag="qtw1")
                nc.vector.tensor_tensor(
                    qtw1.rearrange("p t d -> p (t d)"),
                    qtw.rearrange("p t d -> p (t d)"), mqb, MULT,
                )
                s["qtw1"] = qtw1

            def ph_cprep(b):
                s = S[b]
                cb = big.tile([128, NT1, 128], BF16, tag="cb")
                nc.vector.tensor_copy(cb, s["c1"])
                ct = big.tile([128, NT1, 128], BF16, tag="ct")
                for n in range(4):
                    ps = small.tile([128, 4, 128], BF16, tag="acc")
                    for k in range(4):
                        nc.tensor.transpose(ps[:, k, :], cb[:, 4 * n + k, :], ident_b)
                    nc.vector.tensor_copy(ct[:, 4 * n : 4 * (n + 1), :], ps)
                s["ct"] = ct
                # c1m = [mp_i * c | mp_i | 1]  (mask_p + z2 + z1-colsum operand)
                c1m = big.tile([128, NT1, 130], BF16, tag="c1m")
                for it in range(NT1):
                    nc.vector.tensor_scalar_mul(
                        c1m[:, it, 0:128], cb[:, it, :], s["mp"][:, it : it + 1]
                    )
                nc.gpsimd.tensor_copy(
                    c1m[:, :, 128:129].rearrange("p a b -> p (a b)"), s["mp"]
                )
                nc.gpsimd.memset(c1m[:, :, 129:130], 1.0)
                s["c1m"] = c1m

            def ph_e2(b):
                # e2[i,j] = exp(dot + cwc_i), unmasked (mp applied via c1m)
                s = S[b]
                e2n = big.tile([128, NT1, L2], BF16, tag="e2n")
                for g in range(NT1 // 2):
                    st = strips.tile([128, 2, 512], F32, tag="strip")
                    for k in range(2):
                        nc.tensor.matmul(
                            st[:, k, :], s["ct"][:, 2 * g + k, :],
                            s["qtw"].rearrange("p t d -> p (t d)"),
                            start=True, stop=True,
                        )
                    nc.scalar.activation(e2n[:, 2 * g : 2 * g + 2, :], st, EXP)
                s["e2n"] = e2n

            def ph_e1(b):
                # e1[j,i] = exp(mq_j * (dot + cwc_i)); masked col -> 1 (uniform)
                s = S[b]
                e1 = big.tile([128, NT2, L1], BF16, tag="e1")
                for jt in range(NT2):
                    for h in range(2):
                        st = strips.tile([128, 2, 512], F32, tag="strip")
                        for k in range(2):
                            m = 2 * h + k
                            nc.tensor.matmul(
                                st[:, k, :], s["qtw1"][:, jt, :],
                                s["ct"][:, 4 * m : 4 * (m + 1), :],
                                start=True, stop=True,
                            )
                        nc.scalar.activation(
                            e1[:, jt, 1024 * h : 1024 * (h + 1)],
                            st.rearrange("p a b -> p (a b)"), EXP,
                        )
                s["e1"] = e1

            def ph_t(b):
                s = S[b]
                rhs_ab = work.tile([128, NT2, 256], BF16, tag="rhs_ab")
                for jt in range(NT2):
                    pst = small.tile([128, 130], F32, tag="acc")
                    for it in range(NT1):
                        nc.tensor.matmul(
                            pst, s["e2n"][:, it, jt * 128 : (jt + 1) * 128],
                            s["c1m"][:, it, :],
                            start=(it == 0), stop=(it == NT1 - 1),
                        )
                    # z1_j = mq_j*(colsum_j - L1) + L1 ; colsum in pst[:,129]
                    z1 = work.tile([128, 1], F32, tag="z1")
                    nc.vector.scalar_tensor_tensor(
                        out=z1, in0=pst[:, 129:130], scalar=-float(L1),
                        in1=s["mq"][:, jt : jt + 1], op0=ADD, op1=MULT,
                    )
                    nc.vector.tensor_scalar_add(z1, z1, float(L1))
                    rz1 = work.tile([128, 1], F32, tag="rz1")
                    nc.vector.reciprocal(rz1, z1)
                    rz2 = work.tile([128, 1], F32, tag="rz2")
                    nc.vector.reciprocal(rz2, pst[:, 128:129])
                    rz12 = work.tile([128, 1], F32, tag="rz12")
                    nc.vector.tensor_mul(rz12, rz2, rz1)
                    nc.vector.tensor_scalar_mul(
                        rhs_ab[:, jt, 128:256], pst[:, 0:128], rz12
                    )
                    nc.vector.tensor_scalar_mul(
                        rhs_ab[:, jt, 0:128], s["qnb"][:, jt, :], rz1
                    )
                s["rhs_ab"] = rhs_ab

            def ph_ab(b):
                s = S[b]
                for g in range(NT1 // 4):
                    psab = small.tile([128, 4, 256], F32, tag="acc")
                    # groups must accumulate consecutively: a start=True into a
                    # bank clears has_written for the whole bank
                    for gi in range(4):
                        it = 4 * g + gi
                        for jt in range(NT2):
                            nc.tensor.matmul(
                                psab[:, gi, :],
                                s["e1"][:, jt, it * 128 : (it + 1) * 128],
                                s["rhs_ab"][:, jt, :],
                                start=(jt == 0), stop=(jt == NT2 - 1),
                            )
                    o_sb = outp.tile([128, 4, 384], F32, tag="o_sb")
                    c_sl = s["c1"][:, 4 * g : 4 * (g + 1), :]
                    nc.vector.tensor_copy(o_sb[:, :, 0:128], psab[:, :, 0:128])
                    nc.vector.tensor_tensor(
                        o_sb[:, :, 128:256], c_sl, psab[:, :, 0:128], MULT
                    )
                    nc.vector.tensor_tensor(
                        o_sb[:, :, 256:384], c_sl, psab[:, :, 128:256], MULT
                    )
                    nc.sync.dma_start(
                        out=out_d[b, 512 * g : 512 * (g + 1), 128:512].rearrange(
                            "(t p) m -> p t m", p=128
                        ),
                        in_=o_sb,
                    )

            def ph_dbg(b):
                if not (dbg and b == 0):
                    return
                s = S[b]
                for name, key in [
                    ("dbg_e1", "e1"), ("dbg_e2n", "e2n"),
                    ("dbg_rhs_ab", "rhs_ab"), ("dbg_ct", "ct"), ("dbg_qtw", "qtw"),
                    ("dbg_qtw1", "qtw1"), ("dbg_c1m", "c1m"),
                ]:
                    src = s[key]
                    dd = nc.dram_tensor(
                        name, list(src.shape), src.dtype, kind="ExternalOutput"
                    ).ap()
                    nc.sync.dma_start(out=dd, in_=src)

            # interleaved emission: scheduler always has cross-batch slack
            ph_dma(0); ph_dma(1)
            ph_qprep(0); ph_cprep(0)
            ph_e2(0); ph_qprep(1)
            ph_e1(0); ph_cprep(1)
            ph_t(0); ph_e2(1)
            ph_ab(0); ph_e1(1)
            ph_t(1); ph_ab(1)
            ph_dbg(0)

    nc.compile()
    return nc


_NC = None


def _get_nc():
    global _NC
    if _NC is None:
        _NC = _build_program()
    return _NC


def _make_in_maps(inputs):
    context, query, w = inputs["context"], inputs["query"], inputs["w"]
    w2 = np.asarray(w).reshape(3, D).astype(np.float32)  # rows: wq, wc, wcq
    mp = np.asarray(inputs["mask_p"]).astype(np.float32)  # (B, L1)
    mq = np.asarray(inputs["mask_q"]).astype(np.float32)  # (B, L2)
    # aux[b] = [mp_t (16) | mq_t (4) | w^T (3)] as [128, 23]
    mp_t = mp.reshape(B, NT1, 128).transpose(0, 2, 1)     # (B, 128, 16)
    mq_t = mq.reshape(B, NT2, 128).transpose(0, 2, 1)     # (B, 128, 4)
    wt = np.broadcast_to(w2.T[None], (B, 128, 3))         # (B, 128, 3)
    aux = np.ascontiguousarray(
        np.concatenate([mp_t, mq_t, wt], axis=2), dtype=np.float32
    )
    in_maps = []
    for c in range(NCORES):
        sl = slice(c * BPC, (c + 1) * BPC)
        in_maps.append(
            {
                "context": np.ascontiguousarray(context[sl]),
                "query": np.ascontiguousarray(query[sl]),
                "aux": aux[sl],
                "mq_row": np.ascontiguousarray(mq[sl]),
            }
        )
    return in_maps


def kernel(context, query, w, mask_p, mask_q):
    nc = _get_nc()
    in_maps = _make_in_maps(
        {"context": context, "query": query, "w": w, "mask_p": mask_p, "mask_q": mask_q}
    )
    res = bass_utils.run_bass_kernel_spmd(nc, in_maps, core_ids=list(range(NCORES)))
    return np.concatenate([res.results[c]["out"] for c in range(NCORES)], axis=0)
